# revision 57
# baseline (speedup 1.0000x reference)
"""GWPooling2D forward on 8 Trainium2 NeuronCores.

y[b, c, o] = sum_k m[c, o, k] * x[b, k]   (k = 20*20 input pixels,
o = 16*16 output pixels, c = 16 channels)

The pooling map m (16, 256, 400) depends only on the small `signal`
parameter; it is computed on host (FFTs + 16 complex 576x576 matrix
exponentials), exactly as in the reference.

Device-side structure exploits m = 1 (x) G + dm, where G is the
channel-mean (256, 400) block (the map is near channel-independent:
the shared part carries ~98% of the energy) and dm is the per-channel
correction (~18% of the output norm):

  * G-path: fp16 matmul per batch tile -> s = G x, computed ONCE and
    broadcast-added into all 16 channel blocks.
  * dm-path: fp8(e4m3) DoubleRow matmuls (2 k-values per partition,
    2x PE throughput). fp8 quantization error lands only on the small
    dm component (~0.4% of y).
  * Output is written as fp16 (halves the dominant store traffic; the
    f32 output write was the baseline bottleneck).

PSUM tiles span 2 banks; drains are (128, 1024) wide ops balanced over
three engine paths: DVE fused (psum*alpha + s), ACT scaled-copy -> Pool
fp16 add, and ACT scaled-copy -> DVE fp16 add.
"""

import numpy as np
import ml_dtypes
import scipy.linalg

import concourse.bass as bass
import concourse.bacc as bacc
import concourse.mybir as mybir
import concourse.tile as tile
from concourse.bass_utils import run_bass_kernel_spmd

C = 16
P = (24, 24)
NI = (20, 20)
NO = (16, 16)
B = 8192
NCORES = 8
BS = B // NCORES              # 1024 batch rows per core
K = NI[0] * NI[1]             # 400 contraction
NO2 = NO[0] * NO[1]           # 256 output pixels
O = C * NO2                   # 4096 output features
KA, KB = 128, 72              # fp8 DoubleRow k-tiling: 2*128 + 2*72 = 400
BT = 128                      # batch tile (PSUM partitions)
NPAIR = 4                     # co-pairs per batch tile (each 1024 wide)
XEXP = 5                      # x fp8 scale = 2**XEXP
GEXP = 7                      # G fp8 scale = 2**GEXP

f32 = mybir.dt.float32
f16 = mybir.dt.float16
f8 = mybir.dt.float8e4
u8 = mybir.dt.uint8
DR = mybir.MatmulPerfMode.DoubleRow
E4M3 = ml_dtypes.float8_e4m3

# drain-path assignment per (btile, pair): 1 = DVE fused from PSUM,
# 2 = ACT scaled copy -> Pool fp16 add, 3 = ACT scaled copy -> DVE fp16 add.
# Counts balance engine busy time (DVE 1.26us, ACT 1.04us, Pool 2.13us per
# wide op).
def _make_pattern(counts={1: 14, 2: 11, 3: 7}):
    rem = dict(counts)
    credit = {k: 0.0 for k in counts}
    out = []
    for _ in range(sum(counts.values())):
        for k in counts:
            if rem[k] > 0:
                credit[k] += counts[k]
        k = max((k for k in counts if rem[k] > 0), key=lambda q: credit[q])
        credit[k] -= sum(counts.values())
        rem[k] -= 1
        out.append(k)
    return out


# first (pair-0) sweep is kept light on the slow two-stage paths: its units
# carry the G-stage too and pace the first stores
PATTERN = ([1, 1, 2, 1, 2, 1, 1, 3]
           + _make_pattern({1: 10, 2: 8, 3: 6}))
CONFIG = {"opool": 10, "tpool": 8, "delay": 2, "delay0": 2, "warmup": 6, "hoistg": True, "split": (), "gpst": 0, "gpos": ((1, 1), (3, 2), (5, 3))}


# ---------------------------------------------------------------- host map ---

def _hann(n):
    return 0.5 * (1.0 - np.cos(2.0 * np.pi * np.arange(n) / n))


def _signal_to_spectrum(signal):
    n0, n1 = signal.shape[-2], signal.shape[-1]
    window = _hann(n0)[:, None] * _hann(n1)[None, :]
    rx = np.arange((-n0) // 2 + 1, n0 // 2 + 1)[:, None]
    ry = np.arange((-n1) // 2 + 1, n1 // 2 + 1)[None, :]
    r = (1 + rx * rx + ry * ry).astype(np.float64)
    wf = np.roll(np.fft.fft2(signal), (n0 // 2, n1 // 2), (-2, -1)) / r / 5.0
    wt = np.fft.ifft2(np.roll(wf, (-(n0 // 2), -(n1 // 2)), (-2, -1))) * window
    return np.roll(np.fft.fft2(wt), (n0 // 2, n1 // 2), (-2, -1))


def _gw2d_algebra(w):
    p0, p1 = w.shape[-2], w.shape[-1]
    pad = [(0, 0)] * (w.ndim - 2) + [(p1 // 2, p1 // 2), (p0 // 2, p0 // 2)]
    wp = np.pad(w, pad)
    ia = np.arange(p0)[:, None] + np.arange(p0)[None, :]
    jb = np.arange(p1)[:, None] + np.arange(p1)[None, :]
    ws = wp[..., ia[:, None, :, None], jb[None, :, None, :]]
    ws = ws[..., ::-1, ::-1, :, :]
    kx = np.arange((-p0) // 2 + 1, p0 // 2 + 1)[:, None]
    ky = np.arange((-p1) // 2 + 1, p1 // 2 + 1)[None, :]
    return -1j * (ws[..., 0, :, :, :, :] * kx + ws[..., 1, :, :, :, :] * ky)


def _transform_to_map(t):
    p0, p1 = t.shape[-2], t.shape[-1]
    di = (p0 - NI[0], p1 - NI[1])
    do = (p0 - NO[0], p1 - NO[1])
    x = t[..., do[0] // 2 + 1:(-do[0]) // 2 + 1, do[1] // 2 + 1:(-do[1]) // 2 + 1,
          di[0] // 2 + 1:(-di[0]) // 2 + 1, di[1] // 2 + 1:(-di[1]) // 2 + 1]
    x = np.roll(x, (NO[0] // 2 + 1, NO[1] // 2 + 1, NI[0] // 2 + 1, NI[1] // 2 + 1),
                (-4, -3, -2, -1))
    return np.fft.fft2(np.fft.ifft2(x, axes=(-2, -1)), axes=(-4, -3)).real


def compute_mf(signal):
    """signal (C,2,24,24) -> pooling matrix (O=4096, K=400) float32."""
    spectrum = _signal_to_spectrum(signal.astype(np.float64))
    p0, p1 = spectrum.shape[-2], spectrum.shape[-1]
    a = _gw2d_algebra(spectrum)
    n = p0 * p1
    mat = a.reshape(a.shape[:-4] + (n, n))
    t = np.stack([scipy.linalg.expm(mat[i]) for i in range(mat.shape[0])])
    t = t.reshape(t.shape[:-2] + (p0, p1, p0, p1))
    m = _transform_to_map(t)
    return m.reshape(O, K).astype(np.float32)


def _pack_k(M):
    """(400, N) -> DoubleRow tiles (128, 2, N), (72, 2, N); k = t*KA + p
    for the A tile and 256 + t*KB + p for the B tile."""
    A = np.stack([M[0:128], M[128:256]], axis=1)
    Bt = np.stack([M[256:328], M[328:400]], axis=1)
    return np.ascontiguousarray(A), np.ascontiguousarray(Bt)


def _q8(v, scale):
    return np.clip(v * scale, -240.0, 240.0).astype(E4M3)


# ------------------------------------------------------------ device kernel ---

_cache = {}


def _build(shift):
    key = (shift, tuple(PATTERN), tuple(sorted(CONFIG.items())))
    if key in _cache:
        return _cache[key]
    alpha = float(2.0 ** (-shift))
    nc = bacc.Bacc(dynamic_dma_scratch_size=256)

    # inputs arrive as three contiguous byte blobs per partition group so the
    # critical first-unit set (x16 quad-0 slice + G + x8 + dm pair 0) lands in
    # ONE transfer per group instead of eight HWDGE-setup-paced small ones
    preA_d = nc.declare_dram_parameter("preA", (KA, 5120), u8, isOutput=False)
    preB_d = nc.declare_dram_parameter("preB", (KB, 5120), u8, isOutput=False)
    dm0A_d = nc.declare_dram_parameter("dm0A", (KA, 2048), u8, isOutput=False)
    dm0B_d = nc.declare_dram_parameter("dm0B", (KB, 2048), u8, isOutput=False)
    dmrA_d = nc.declare_dram_parameter("dmrA", (KA, 6144), u8, isOutput=False)
    dmrB_d = nc.declare_dram_parameter("dmrB", (KB, 6144), u8, isOutput=False)
    out_d = nc.declare_dram_parameter("out", (BS, O), f16, isOutput=True)

    with tile.TileContext(nc) as tc:
        with tc.tile_pool(name="wpool", bufs=1) as wpool, \
             tc.tile_pool(name="spool", bufs=1) as spool, \
             tc.tile_pool(name="tpool", bufs=CONFIG["tpool"]) as tpool, \
             tc.tile_pool(name="opool", bufs=CONFIG["opool"]) as opool, \
             tc.tile_pool(name="dps", bufs=4, space="PSUM") as dps:
            preAt = wpool.tile([KA, 5120], u8, name="preAt")
            preBt = wpool.tile([KB, 5120], u8, name="preBt")
            dm0At = wpool.tile([KA, 2048], u8, name="dm0At")
            dm0Bt = wpool.tile([KB, 2048], u8, name="dm0Bt")
            dmrAt = wpool.tile([KA, 6144], u8, name="dmrAt")
            dmrBt = wpool.tile([KB, 6144], u8, name="dmrBt")

            def views(pre, dm0, dmr):
                t2 = lambda ap, dt: ap.bitcast(dt).rearrange(
                    "p (t q) -> p t q", t=2)
                return (t2(pre[:, 0:512], f8),        # Ghi (·,2,256)
                        t2(pre[:, 512:1024], f8),     # Glo (·,2,256)
                        t2(pre[:, 1024:3072], f8),    # x8 hi (·,2,1024)
                        t2(pre[:, 3072:5120], f8),    # x8 lo (·,2,1024)
                        t2(dm0[:], f8),               # dm pair 0 (·,2,1024)
                        t2(dmr[:], f8))               # dm rest (·,2,3072)
            ghiA, gloA, x8A, xloA, dm0A, dmrA = views(preAt, dm0At, dmrAt)
            ghiB, gloB, x8B, xloB, dm0B, dmrB = views(preBt, dm0Bt, dmrBt)

            # PE p-state warm-up: the tensor engine only reaches full
            # clock after ~3us of gapless work, and the G+pair sweep must
            # run at full speed to keep the store stream fed. Chain dummy
            # matmuls (on memset data, result never read) from t~0.6us so
            # the PE enters the real G-phase already ramped.
            if CONFIG["warmup"]:
                dwm = wpool.tile([BT, 128], f16, name="dwm")
                dmv = wpool.tile([BT, 512], f16, name="dmv")
                nc.gpsimd.memset(dwm[:], 0.0)
                nc.gpsimd.memset(dmv[:], 0.0)
                wps = dps.tile([BT, 1024], f32, name="ps", tag="ps")
                for w in range(CONFIG["warmup"]):
                    nc.tensor.matmul(wps[:, 0:512], dwm[:], dmv[:],
                                     start=True, stop=True)

            # loads ordered so the first unit (pair 0, btile 0) can start
            # ASAP: weights + x8 + dm pair 0 + just the first 128-batch slice
            # of x16; the rest streams in behind
            nc.sync.dma_start(preAt[:], preA_d[:])
            nc.sync.dma_start(preBt[:], preB_d[:])
            nc.sync.dma_start(dm0At[:], dm0A_d[:])
            nc.sync.dma_start(dm0Bt[:], dm0B_d[:])
            nc.sync.dma_start(dmrAt[:, 0:2048], dmrA_d[:, 0:2048])
            nc.sync.dma_start(dmrBt[:, 0:2048], dmrB_d[:, 0:2048])
            nc.sync.dma_start(dmrAt[:, 2048:6144], dmrA_d[:, 2048:6144])
            nc.sync.dma_start(dmrBt[:, 2048:6144], dmrB_d[:, 2048:6144])

            # pair-major dm matmuls with quarter-row stores; the shared
            # G-path s_b = G x_b (fp16; folded residual lives in dm) is
            # interleaved into the first pair sweep and the s_b stay in SBUF
            # for the broadcast-adds of all later pairs. Pool-path stores are
            # delayed two units in the in-order SP queue so their slower
            # drains don't block neighbours' stores.
            sbcs = []
            pending = []   # (release_at_unit, dst_ap, src_ap)

            nstores = [0]

            def flush(now):
                for ent in sorted(pending, key=lambda e: e[0]):
                    if ent[0] <= now:
                        if nstores[0] < CONFIG["gpst"]:
                            nc.gpsimd.dma_start(ent[1], ent[2])
                        else:
                            nc.sync.dma_start(ent[1], ent[2])
                        nstores[0] += 1
                        pending.remove(ent)

            aG = float(2.0 ** (-(GEXP + XEXP)))

            def emit_g(q):
                # G-path in double-fp8: fp8 is a float, so the residual
                # terms (Ghi.xlo, Glo.xhi) carry the SAME product scale as
                # Ghi.xhi when packed at the same exponents — all three
                # accumulate in one PSUM group, drained by a single wide
                # ACT scaled copy (lo.lo term ~6e-5 rel is dropped; G's fp8
                # residual is folded into dm on the host)
                s_pw = dps.tile([BT, 1024], f32, name="ps", tag="ps")
                for i in (0, 1):
                    gb = 2 * q + i
                    bsl2 = slice(gb * BT, (gb + 1) * BT)
                    s_ps = s_pw[:, i * NO2:(i + 1) * NO2]
                    nc.tensor.matmul(s_ps, x8A[:, :, bsl2], ghiA[:],
                                     perf_mode=DR, start=True, stop=False)
                    nc.tensor.matmul(s_ps, x8B[:, :, bsl2], ghiB[:],
                                     perf_mode=DR, start=False, stop=False)
                    nc.tensor.matmul(s_ps, xloA[:, :, bsl2], ghiA[:],
                                     perf_mode=DR, start=False, stop=False)
                    nc.tensor.matmul(s_ps, xloB[:, :, bsl2], ghiB[:],
                                     perf_mode=DR, start=False, stop=False)
                    nc.tensor.matmul(s_ps, x8A[:, :, bsl2], gloA[:],
                                     perf_mode=DR, start=False, stop=False)
                    nc.tensor.matmul(s_ps, x8B[:, :, bsl2], gloB[:],
                                     perf_mode=DR, start=False, stop=True)
                s_sb = spool.tile([BT, 2, NO2], f16, name=f"s_sb{q}")
                nc.scalar.mul(s_sb[:].rearrange("p a q -> p (a q)"),
                              s_pw[:, 0:2 * NO2], aG)
                for i in (0, 1):
                    sbcs.append(s_sb[:, i, :].unsqueeze(1)
                                .broadcast_to([BT, 4, NO2]))

            if CONFIG["hoistg"]:
                emit_g(0)

            for j in range(NPAIR):
                for b in range(BS // BT):
                    bsl = slice(b * BT, (b + 1) * BT)
                    if CONFIG["hoistg"]:
                        gq = dict(CONFIG["gpos"])
                        if j == 0 and b in gq:
                            emit_g(gq[b])
                    elif j == 0 and b % 2 == 0:
                        emit_g(b // 2)
                    ps = dps.tile([BT, 1024], f32, name="ps", tag="ps")
                    dA = dm0A if j == 0 else dmrA
                    dB = dm0B if j == 0 else dmrB
                    joff = 0 if j == 0 else (j - 1) * 1024
                    for h in range(2):
                        csl = slice(joff + h * 512, joff + h * 512 + 512)
                        psl = slice(h * 512, (h + 1) * 512)
                        nc.tensor.matmul(ps[:, psl], x8A[:, :, bsl],
                                         dA[:, :, csl], perf_mode=DR,
                                         start=True, stop=False)
                        nc.tensor.matmul(ps[:, psl], x8B[:, :, bsl],
                                         dB[:, :, csl], perf_mode=DR,
                                         start=False, stop=True)
                    unit = j * (BS // BT) + b
                    path = PATTERN[unit]
                    halves = ((0, 2), (2, 4)) if unit in CONFIG["split"] \
                        else ((0, 4),)
                    for h0, h1 in halves:
                        w = (h1 - h0) * NO2
                        qt = opool.tile([BT, 1024], f16, name="qt")
                        osl = qt[:, 0:w].rearrange("p (a q) -> p a q",
                                                   a=h1 - h0)
                        psr = ps[:, h0 * NO2:h1 * NO2].rearrange(
                            "p (a q) -> p a q", a=h1 - h0)
                        sb_b = sbcs[b] if h1 - h0 == 4 else \
                            sbcs[b][:, h0:h1, :]
                        if path == 1:
                            nc.vector.scalar_tensor_tensor(
                                osl, psr, alpha, sb_b,
                                op0=mybir.AluOpType.mult,
                                op1=mybir.AluOpType.add)
                        else:
                            tmp = tpool.tile([BT, 1024], f16, name="tmp")
                            tms = tmp[:, 0:w]
                            nc.scalar.mul(tms, ps[:, h0 * NO2:h1 * NO2], alpha)
                            tmr = tms.rearrange("p (a q) -> p a q", a=h1 - h0)
                            if path == 2:
                                nc.gpsimd.tensor_add(osl, tmr, sb_b)
                            else:
                                nc.vector.tensor_add(osl, tmr, sb_b)
                        d = CONFIG["delay0"] if unit < 8 else CONFIG["delay"]
                        release = unit + d if path == 2 else unit
                        pending.append(
                            (release,
                             out_d[bsl, j * 1024 + h0 * NO2:
                                   j * 1024 + h1 * NO2],
                             qt[:, 0:w]))
                    flush(unit)
            flush(10 ** 9)
    nc.compile()
    _cache[key] = nc
    return nc


_prepared = None


def _prepare(x, signal):
    """Host precompute: pooling map, split, quantization, packing."""
    global _prepared
    mf = compute_mf(np.asarray(signal))          # (4096, 400)
    m = mf.reshape(C, NO2, K)
    G = m.mean(axis=0)                           # channel-shared part
    # double-fp8 G; its quantization residual folds into dm exactly
    Ghi = _q8(G, 2.0 ** GEXP)
    Glo = _q8(G - Ghi.astype(np.float32) / 2.0 ** GEXP, 2.0 ** GEXP)
    Geff = (Ghi.astype(np.float32) + Glo.astype(np.float32)) / 2.0 ** GEXP
    dm = (m - Geff[None]).reshape(O, K)

    dmax = float(np.abs(dm).max())
    dexp = int(np.floor(np.log2(200.0 / max(dmax, 1e-30))))
    shift = dexp + XEXP

    ghA, ghB = _pack_k(np.ascontiguousarray(Ghi.T))      # (400, 256) fp8
    glA, glB = _pack_k(np.ascontiguousarray(Glo.T))
    dmA, dmB = _pack_k(_q8(dm.T, 2.0 ** dexp))           # (400, 4096) fp8

    def _blob(parts):
        rows = [np.ascontiguousarray(a).view(np.uint8).reshape(a.shape[0], -1)
                for a in parts]
        return np.ascontiguousarray(np.concatenate(rows, axis=1))

    xf = np.asarray(x).reshape(B, K)
    in_maps = []
    for i in range(NCORES):
        xT = np.ascontiguousarray(xf[i * BS:(i + 1) * BS].T)  # (400, 1024)
        x8A, x8B = _pack_k(_q8(xT, 2.0 ** XEXP))
        rT = xT - (np.concatenate([x8A[:, 0], x8A[:, 1], x8B[:, 0],
                                   x8B[:, 1]]).astype(np.float32)
                   / 2.0 ** XEXP)
        xlA, xlB = _pack_k(_q8(rT, 2.0 ** XEXP))
        in_maps.append({
            "preA": _blob([ghA, glA, x8A, xlA]),
            "preB": _blob([ghB, glB, x8B, xlB]),
            "dm0A": _blob([dmA[:, :, 0:1024]]),
            "dm0B": _blob([dmB[:, :, 0:1024]]),
            "dmrA": _blob([dmA[:, :, 1024:O]]),
            "dmrB": _blob([dmB[:, :, 1024:O]]),
        })
    _prepared = (in_maps, shift)
    return in_maps, shift


def _run(x, signal, **spmd_kwargs):
    in_maps, shift = _prepare(x, signal)
    nc = _build(shift)
    res = run_bass_kernel_spmd(nc, in_maps, list(range(NCORES)), **spmd_kwargs)
    y = np.concatenate(
        [r["out"].astype(np.float32) for r in res.results], axis=0)
    return y.reshape(B, C, NO[0], NO[1]), res


def kernel(x, signal):
    y, _ = _run(x, signal, trace=False)
    return y


def _built_nc():
    """Most recently built module (for TimelineSim in test.py)."""
    assert _cache, "kernel not built yet"
    return next(iter(_cache.values()))
